# revision 11
# baseline (speedup 1.0000x reference)
"""Trainium2 Bass kernel for nn_IntraCycleMoELayer (MoE routing, 8 cores).

Strategy
--------
The reference computes all E=8 experts densely, but the top-2 gate zeroes all
but 2 experts per batch row.  Work that matters: per row, the top-1 routed
expert (gate >= 0.5 by softmax-renorm structure), the top-2 expert (often with
a near-zero gate), and the general expert, each an MLP block
  LN(gelu_tanh(x @ w1 + b1) @ w2 + b2 + x) * gamma + beta
over [L=512 tokens, D=768], DFF=3072.

Precision (for the 2e-2 rel-err budget; fp8e4 matmuls use DoubleRow perf
mode = 2x PE throughput, contracting K=256/instr):
  - general + top-1 expert ("M2"): mm1 fp16, gelu -> fp8 h, mm2 fp8
    DoubleRow.  The w2 absmax scale is folded into the residual (LN is
    scale-invariant up to eps, which is corrected via the Sqrt bias).
  - top-2 expert with gate >= 0.01 ("F8"): both matmuls fp8 DoubleRow; mm1
    dequant folds into the gelu activation's scale operand.
  - top-2 expert with gate < 0.01: skipped (contributes < 1e-3 rel).

When gamma is uniform and beta is zero (as in this model), the gate and
gamma fold into the LN rstd via the Sqrt activation's scale operand, so the
LN tail is a single tensor_scalar writing fp16 output.

Layout: core c owns rows 2c, 2c+1 -> 4 full M2 slots (general x2, top-1 x2);
the F8 jobs are split into 256-token halves spread 1 per core and run FIRST
(cheap fp8 weights shorten the critical head DMA); its mm2 is emitted after
the first M2 slot's mm1 so the gelu drain overlaps.  Host sums per-row
outputs (general + bf16(sum of routed)) exactly as the reference does.
"""
import numpy as np
import ml_dtypes

import concourse.bass as bass
import concourse.mybir as mybir
import concourse.tile as tile
from concourse import bacc
from concourse.bass import ts
from concourse import bass_utils

B, L, D, DFF, DLLM, E, TOPK = 16, 512, 768, 3072, 4096, 8, 2
EPS_GATE = 1e-9
LN_EPS = 1e-5
NCORES = 8
RPC = 2                               # rows per core
KC1, MC1 = D // 128, DFF // 128       # 6, 24
KC2, TC = DFF // 128, L // 128        # 24, 4
HL = L // 2                           # 256-token half
CB = DFF // 4                         # 768-column DMA piece
F8NP = ml_dtypes.float8_e4m3          # TRN float8e4 (max 240)
F8CAP = 240.0
SKIP_G = 0.01
dt = mybir.dt

_cache = {}   # (nf8, m2_reload, fold) -> finalized nc


def _router(cycle_numbers, DKP_embeddings, gate_We, gate_Wc, gate_b, gate_Wo,
            gate_bo):
    h = np.maximum(
        DKP_embeddings @ gate_We + cycle_numbers @ gate_Wc + gate_b, 0.0)
    logits = h @ gate_Wo + gate_bo                       # [B, E]
    idx = np.argsort(-logits, axis=1, kind="stable")[:, :TOPK]
    m = logits.max(axis=1, keepdims=True)
    p = np.exp(logits - m)
    p /= p.sum(axis=1, keepdims=True)
    mask = np.zeros_like(p)
    mask[np.arange(logits.shape[0])[:, None], idx] = 1.0
    gated = p * mask
    gated = gated / (gated.sum(axis=1, keepdims=True) + EPS_GATE)
    return idx, gated


def _slot_kinds(nf8, m2_reload):
    """Slot order shared by program build and host staging.

    Returns list of kinds: "F8" (half job, fp8 mm1+mm2) or "M2" (full job,
    fp16 mm1 + fp8 mm2).  w1a sets: [GEN, e6(row0), e6(row1) if reload].
    w2b sets are consumed in slot order by every slot's load.
    """
    kinds = []
    if nf8 >= 1:
        kinds.append("F8")
    kinds += ["M2", "M2", "M2", "M2"]
    kinds += ["F8"] * (nf8 - 1)
    return kinds


def _build_nc(nf8, m2_reload, fold):
    key = (nf8, m2_reload, fold)
    if key in _cache:
        return _cache[key]

    kinds = _slot_kinds(nf8, m2_reload)
    NSLOT = len(kinds)
    n_w1a = 2 + (1 if m2_reload else 0)
    # w2b sets in slot order: one per F8 slot, one for GEN (first M2),
    # one (or two with reload) for e6.
    n_w2b = nf8 + 2 + (1 if m2_reload else 0)
    n_w1b = max(nf8, 1)
    n_xtb = max(nf8, 1)
    NYH = 8 + nf8

    nc = bacc.Bacc("TRN2", target_bir_lowering=False, debug=False)
    # all staged pre-arranged partition-major so DMA lines are 3-18KB
    w1a_d = nc.dram_tensor("w1a", [n_w1a, 128, 4, KC1, CB], dt.float16, kind="ExternalInput")
    w1b_d = nc.dram_tensor("w1b", [n_w1b, 128, 4, KC1, CB], dt.float8e4, kind="ExternalInput")
    w2b_d = nc.dram_tensor("w2b", [n_w2b, 128, KC2, D], dt.float8e4, kind="ExternalInput")
    xTa_d = nc.dram_tensor("xTa", [RPC, 128, KC1, L], dt.float16, kind="ExternalInput")
    xTb_d = nc.dram_tensor("xTb", [n_xtb, 128, KC1, HL], dt.float8e4, kind="ExternalInput")
    xr_d = nc.dram_tensor("xr", [NYH, 128, 2, D], dt.float16, kind="ExternalInput")
    b1_d = nc.dram_tensor("b1", [128, NSLOT, MC1], dt.float32, kind="ExternalInput")
    as_d = nc.dram_tensor("acts", [128, NSLOT, 3], dt.float32, kind="ExternalInput")
    gb_d = nc.dram_tensor("gb", [NSLOT, 2, D], dt.float16, kind="ExternalInput")
    y_d = nc.dram_tensor("y", [NYH, 128, 2, D], dt.float16, kind="ExternalOutput")

    gelu = mybir.ActivationFunctionType.Gelu_apprx_tanh
    DR = mybir.MatmulPerfMode.DoubleRow

    with tile.TileContext(nc) as tc, \
         tc.tile_pool(name="w1ap", bufs=2) as w1ap, \
         tc.tile_pool(name="w1bp", bufs=1) as w1bp, \
         tc.tile_pool(name="w2bp", bufs=2) as w2bp, \
         tc.tile_pool(name="xtap", bufs=RPC) as xtap, \
         tc.tile_pool(name="xtbp", bufs=1) as xtbp, \
         tc.tile_pool(name="h8p", bufs=2) as h8p, \
         tc.tile_pool(name="xrp", bufs=3) as xrp, \
         tc.tile_pool(name="gbp", bufs=2) as gbp, \
         tc.tile_pool(name="rp", bufs=3) as rp, \
         tc.tile_pool(name="yp", bufs=3) as yp, \
         tc.tile_pool(name="sp", bufs=4) as sp, \
         tc.tile_pool(name="cp", bufs=1) as cp, \
         tc.tile_pool(name="php", bufs=4, space="PSUM") as php, \
         tc.tile_pool(name="pop", bufs=2, space="PSUM") as pop:

        # ---------- prologue: small loads + PE warmup ----------
        b1_all = cp.tile([128, NSLOT, MC1], dt.float32)
        nc.gpsimd.dma_start(b1_all, b1_d[:])
        as_all = cp.tile([128, NSLOT, 3], dt.float32)
        nc.gpsimd.dma_start(as_all, as_d[:])

        warm_z = cp.tile([128, 512], dt.float16)
        nc.vector.memset(warm_z, 0.0)
        for _ in range(16):
            wp_t = php.tile([128, 512], dt.float32, tag="ph")
            nc.tensor.matmul(wp_t, lhsT=warm_z[:, 0:128], rhs=warm_z,
                             start=True, stop=True)

        # ---------- prologue: critical-path weight streams ----------
        # sync queue: F8 weights first (small), then the second half of w1a
        # GEN + xT row1 + w1a e6 + w2b e6.
        # gpsimd queue: xT row0 + first half of w1a GEN + w2b e4/GEN.
        xtb_sb = None
        if nf8 >= 1:
            xtb_sb = xtbp.tile([128, KC1, HL], dt.float8e4, tag="xtb")
            nc.sync.dma_start(xtb_sb, xTb_d[0])
            w1b_sb = w1bp.tile([128, 4, KC1, CB], dt.float8e4, tag="w1b")
            for cb in range(0, 4, 2):
                nc.sync.dma_start(w1b_sb[:, cb:cb + 2], w1b_d[0][:, cb:cb + 2])

        xT_sb = []
        for _ in range(RPC):
            t_ = xtap.tile([128, KC1, L], dt.float16, tag="xT")
            xT_sb.append(t_)
        nc.gpsimd.dma_start(xT_sb[0], xTa_d[0])

        w1a_gen = w1ap.tile([128, 4, KC1, CB], dt.float16, tag="w1a")
        for cb in range(2):
            nc.gpsimd.dma_start(w1a_gen[:, cb], w1a_d[0][:, cb])
        for cb in range(2, 4):
            nc.sync.dma_start(w1a_gen[:, cb], w1a_d[0][:, cb])
        nc.sync.dma_start(xT_sb[1], xTa_d[1])

        w1a_e6 = [None, None]
        w1a_e6[0] = w1ap.tile([128, 4, KC1, CB], dt.float16, tag="w1a", name="w1a_e6a")
        for cb in range(0, 4, 2):
            nc.sync.dma_start(w1a_e6[0][:, cb:cb + 2], w1a_d[1][:, cb:cb + 2])
        if m2_reload:
            # Loaded lazily at its slot so the ring-WAR on the GEN buffer
            # doesn't block the sync queue head.
            w1a_e6[1] = None
        else:
            w1a_e6[1] = w1a_e6[0]

        # w2b sets, consumed in slot order.  e4 + GEN early on gpsimd; e6
        # late on sync (ring WAR on the e4 buffer resolves before needed).
        w2b_tiles = []
        n_w2b_emitted = 0

        def load_w2b(queue):
            nonlocal n_w2b_emitted
            t_ = w2bp.tile([128, KC2, D], dt.float8e4, tag="w2b")
            src = w2b_d[n_w2b_emitted]
            queue.dma_start(t_[:, 0:12, :], src[:, 0:12, :])
            queue.dma_start(t_[:, 12:KC2, :], src[:, 12:KC2, :])
            n_w2b_emitted += 1
            w2b_tiles.append(t_)
            return t_

        if nf8 >= 1:
            load_w2b(nc.gpsimd)            # e4 (slot 0)
        load_w2b(nc.gpsimd)                # GEN (M2 slots 0-1)
        load_w2b(nc.sync)                  # e6 (first M2e slot)
        if m2_reload:
            load_w2b(nc.sync)              # e6 second row

        kinds_ex = []   # (kind, slot_idx, yidx, row_or_f8idx, w1a_tile, w2b_tile)
        yidx = 0
        m2_seen = 0
        f8_seen = 0
        w2b_order = []
        if nf8 >= 1:
            w2b_order.append(0)
        w2b_order += [1 if nf8 >= 1 else 0] * 2
        base_e6 = (2 if nf8 >= 1 else 1)
        w2b_order += [base_e6, base_e6 + (1 if m2_reload else 0)]

        slot_descs = []
        for s, kind in enumerate(_slot_kinds(nf8, m2_reload)):
            if kind == "M2":
                row = m2_seen % RPC
                late = m2_seen == 3 and m2_reload
                w1a_t = w1a_gen if m2_seen < 2 else w1a_e6[row]
                w2b_t = w2b_tiles[w2b_order[s]]
                slot_descs.append(dict(kind=kind, s=s, yidx=yidx, row=row,
                                       w1a=w1a_t, w2b=w2b_t, late=late))
                m2_seen += 1
                yidx += 2
            else:
                slot_descs.append(dict(kind=kind, s=s, yidx=yidx, f8=f8_seen))
                f8_seen += 1
                yidx += 1

        # ---------- compute emission ----------
        def emit_mm1(sd):
            s = sd["s"]
            b1_sb = b1_all[:, s, :]
            gsc = as_all[:, s, 0:1]
            h8 = h8p.tile([128, KC2, L], dt.float8e4, tag="h8")
            sd["h8"] = h8
            if sd["kind"] == "F8":
                f = sd["f8"]
                if f == 0:
                    w1b_t, xtb_t = w1b_sb, xtb_sb
                else:
                    w1b_t = w1bp.tile([128, 4, KC1, CB], dt.float8e4, tag="w1b")
                    nc.sync.dma_start(w1b_t, w1b_d[f])
                    xtb_t = xtbp.tile([128, KC1, HL], dt.float8e4, tag="xtb")
                    nc.sync.dma_start(xtb_t, xTb_d[f])
                    sd["w2b"] = load_w2b(nc.gpsimd)
                for m in range(MC1):
                    ph = php.tile([128, 512], dt.float32, tag="ph")
                    cb, mc = m // 6, (m % 6) * 128
                    for k2 in range(KC1 // 2):
                        nc.tensor.matmul(
                            ph[:, 0:HL],
                            lhsT=w1b_t[:, cb, 2 * k2:2 * k2 + 2, mc:mc + 128],
                            rhs=xtb_t[:, 2 * k2:2 * k2 + 2, :],
                            start=(k2 == 0), stop=(k2 == KC1 // 2 - 1),
                            perf_mode=DR)
                    nc.scalar.activation(out=h8[:, m, 0:HL], in_=ph[:, 0:HL],
                                         func=gelu, bias=b1_sb[:, m:m + 1],
                                         scale=gsc)
            else:
                if sd.get("late") and sd["w1a"] is None:
                    w1a_t = w1ap.tile([128, 4, KC1, CB], dt.float16, tag="w1a")
                    for cb in range(0, 4, 2):
                        nc.sync.dma_start(w1a_t[:, cb:cb + 2],
                                          w1a_d[2][:, cb:cb + 2])
                    sd["w1a"] = w1a_t
                w1a_t = sd["w1a"]
                xT_t = xT_sb[sd["row"]]
                for m in range(MC1):
                    ph = php.tile([128, 512], dt.float32, tag="ph")
                    cb, mc = m // 6, (m % 6) * 128
                    for k in range(KC1):
                        nc.tensor.matmul(
                            ph, lhsT=w1a_t[:, cb, k, mc:mc + 128],
                            rhs=xT_t[:, k, :],
                            start=(k == 0), stop=(k == KC1 - 1))
                    nc.scalar.activation(out=h8[:, m, :], in_=ph,
                                         func=gelu, bias=b1_sb[:, m:m + 1],
                                         scale=gsc)

        def emit_mm2_ln(sd):
            s = sd["s"]
            h8 = sd["h8"]
            if sd["kind"] == "F8" and sd["f8"] == 0:
                w2b_t = w2b_tiles[0]
            else:
                w2b_t = sd["w2b"]
            nyh = 2 if sd["kind"] == "M2" else 1
            xr_sb = []
            for hh in range(nyh):
                t_ = xrp.tile([128, 2, D], dt.float16, tag="xr")
                nc.gpsimd.dma_start(t_, xr_d[sd["yidx"] + hh])
                xr_sb.append(t_)
            gb_sb = None
            if not fold:
                gb_sb = gbp.tile([128, 2, D], dt.float16, tag="gb")
                gb_ap = gb_d[s]
                nc.gpsimd.dma_start(gb_sb, bass.AP(tensor=gb_ap.tensor,
                                                   offset=gb_ap.offset,
                                                   ap=[[0, 128], *gb_ap.ap]))
            ntc = TC if sd["kind"] == "M2" else TC // 2
            for t in range(ntc):
                po = pop.tile([128, D], dt.float32, tag="po")
                for k2 in range(KC2 // 2):
                    nc.tensor.matmul(
                        po[:, 0:512],
                        lhsT=h8[:, 2 * k2:2 * k2 + 2, ts(t, 128)],
                        rhs=w2b_t[:, 2 * k2:2 * k2 + 2, 0:512],
                        start=(k2 == 0), stop=(k2 == KC2 // 2 - 1),
                        perf_mode=DR)
                    nc.tensor.matmul(
                        po[:, 512:D],
                        lhsT=h8[:, 2 * k2:2 * k2 + 2, ts(t, 128)],
                        rhs=w2b_t[:, 2 * k2:2 * k2 + 2, 512:D],
                        start=(k2 == 0), stop=(k2 == KC2 // 2 - 1),
                        perf_mode=DR)
                r_sb = rp.tile([128, D], dt.float32, tag="r")
                nc.vector.tensor_add(r_sb, po, xr_sb[t // 2][:, t % 2, :])
                stats = sp.tile([128, 3, 6], dt.float32, tag="st")
                for s3 in range(3):
                    nc.vector.bn_stats(stats[:, s3, :], r_sb[:, ts(s3, 256)])
                mv = sp.tile([128, 2], dt.float32, tag="mv")
                nc.vector.bn_aggr(mv, stats)
                rstd = sp.tile([128, 1], dt.float32, tag="rstd")
                nc.scalar.activation(out=rstd, in_=mv[:, 1:2],
                                     func=mybir.ActivationFunctionType.Sqrt,
                                     bias=as_all[:, s, 2:3],
                                     scale=as_all[:, s, 1:2])
                nc.vector.reciprocal(rstd, rstd)
                if t % 2 == 0:
                    yh16 = yp.tile([128, 2, D], dt.float16, tag="y16")
                y16 = yh16[:, t % 2, :]
                if fold:
                    nc.vector.tensor_scalar(out=y16, in0=r_sb,
                                            scalar1=mv[:, 0:1], scalar2=rstd,
                                            op0=mybir.AluOpType.subtract,
                                            op1=mybir.AluOpType.mult)
                else:
                    nc.vector.tensor_scalar(out=r_sb, in0=r_sb,
                                            scalar1=mv[:, 0:1], scalar2=rstd,
                                            op0=mybir.AluOpType.subtract,
                                            op1=mybir.AluOpType.mult)
                    nc.vector.tensor_mul(r_sb, r_sb, gb_sb[:, 0, :])
                    nc.vector.tensor_add(y16, r_sb, gb_sb[:, 1, :])
                if t % 2 == 1:
                    nc.sync.dma_start(y_d[sd["yidx"] + t // 2], yh16)

        if nf8 >= 1:
            # F8 mm1 first (cheap weights), then first M2 mm1 so the PE is
            # fed while the F8 gelu drains; F8 mm2 slots in after.
            emit_mm1(slot_descs[0])
            emit_mm1(slot_descs[1])
            emit_mm2_ln(slot_descs[0])
            emit_mm2_ln(slot_descs[1])
            rest = slot_descs[2:]
        else:
            rest = slot_descs
        for sd in rest:
            emit_mm1(sd)
            emit_mm2_ln(sd)

    nc.finalize()
    _cache[key] = nc
    return nc


def _pm(a, kchunks):
    """[K*128, N] -> [128, K, N] partition-major."""
    return np.ascontiguousarray(
        a.reshape(kchunks, 128, a.shape[-1]).transpose(1, 0, 2))


def _pm_w1(a):
    """[D, DFF] -> [128, 4(cb), KC1, CB] partition-major, cb-blocked."""
    return np.ascontiguousarray(
        a.reshape(KC1, 128, 4, CB).transpose(1, 2, 0, 3))


def kernel(cycle_curve_data, cycle_numbers, DKP_embeddings,
           gate_We, gate_Wc, gate_b, gate_Wo, gate_bo,
           e_w1, e_b1, e_w2, e_b2, e_gamma, e_beta,
           g_w1, g_b1, g_w2, g_b2, g_gamma, g_beta):
    x = np.asarray(cycle_curve_data, dtype=np.float32)
    idx, gated = _router(np.asarray(cycle_numbers, np.float32),
                         np.asarray(DKP_embeddings, np.float32),
                         np.asarray(gate_We, np.float32),
                         np.asarray(gate_Wc, np.float32),
                         np.asarray(gate_b, np.float32),
                         np.asarray(gate_Wo, np.float32),
                         np.asarray(gate_bo, np.float32))

    GEN = E
    w1s = {**{e: np.asarray(e_w1[e], np.float32) for e in range(E)},
           GEN: np.asarray(g_w1, np.float32)}
    w2s = {**{e: np.asarray(e_w2[e], np.float32) for e in range(E)},
           GEN: np.asarray(g_w2, np.float32)}
    b1s = {**{e: np.asarray(e_b1[e], np.float32) for e in range(E)},
           GEN: np.asarray(g_b1, np.float32)}
    b2s = {**{e: np.asarray(e_b2[e], np.float32) for e in range(E)},
           GEN: np.asarray(g_b2, np.float32)}
    gms = {**{e: np.asarray(e_gamma[e], np.float32) for e in range(E)},
           GEN: np.asarray(g_gamma, np.float32)}
    bts = {**{e: np.asarray(e_beta[e], np.float32) for e in range(E)},
           GEN: np.asarray(g_beta, np.float32)}

    # job classification (top-1 always has gate >= 0.5 -> M2; top-2 skipped
    # below SKIP_G, else F8 halves)
    m2_jobs = [(r, int(idx[r, 0]), float(gated[r, idx[r, 0]]))
               for r in range(B)]
    f8_jobs = [(r, int(idx[r, 1]), float(gated[r, idx[r, 1]]))
               for r in range(B) if gated[r, idx[r, 1]] >= SKIP_G]
    f8_halves = [(r, e, g, h) for (r, e, g) in f8_jobs for h in (0, 1)]
    nf8 = (len(f8_halves) + NCORES - 1) // NCORES
    m2_reload = any(m2_jobs[2 * c][1] != m2_jobs[2 * c + 1][1]
                    for c in range(NCORES))

    used_sets = {GEN} | {e for _, e, _ in m2_jobs} | {e for _, e, _ in f8_jobs}
    fold = all(
        np.all(gms[s] == gms[s].flat[0]) and gms[s].flat[0] > 0
        and np.all(bts[s] == 0.0) for s in used_sets)

    nc = _build_nc(nf8, m2_reload, fold)

    kinds = _slot_kinds(nf8, m2_reload)
    NSLOT = len(kinds)
    n_w1a = 2 + (1 if m2_reload else 0)
    n_w2b = nf8 + 2 + (1 if m2_reload else 0)
    n_w1b = max(nf8, 1)
    n_xtb = max(nf8, 1)
    NYH = 8 + nf8

    f16w, q8w = {}, {}

    def w16(s):
        if s not in f16w:
            f16w[s] = _pm_w1(w1s[s].astype(np.float16))
        return f16w[s]

    def w8(kind, s):
        if (kind, s) not in q8w:
            w = w1s[s] if kind == 1 else w2s[s]
            sc = F8CAP / max(float(np.abs(w).max()), 1e-30)
            q = (w * sc).astype(F8NP)
            q = _pm_w1(q) if kind == 1 else _pm(q, KC2)
            q8w[(kind, s)] = (q, sc)
        return q8w[(kind, s)]

    f8_by_core = [[] for _ in range(NCORES)]
    for i, hf in enumerate(f8_halves):
        f8_by_core[i % NCORES].append(hf)

    in_maps = []
    slot_tables = []   # per core: list of (kind, row, expert, half, dummy)
    for c in range(NCORES):
        rows = [RPC * c + i for i in range(RPC)]
        w1a_st = np.empty((n_w1a, 128, 4, KC1, CB), np.float16)
        w1a_st[0] = w16(GEN)
        w1a_st[1] = w16(m2_jobs[rows[0]][1])
        if m2_reload:
            w1a_st[2] = w16(m2_jobs[rows[1]][1])
        w2b_st = np.zeros((n_w2b, 128, KC2, D), F8NP)
        w1b_st = np.zeros((n_w1b, 128, 4, KC1, CB), F8NP)
        xtb_st = np.zeros((n_xtb, 128, KC1, HL), F8NP)
        xr_st = np.zeros((NYH, 128, 2, D), np.float16)
        b1_st = np.zeros((128, NSLOT, MC1), np.float32)
        as_st = np.ones((128, NSLOT, 3), np.float32)
        as_st[:, :, 2] = LN_EPS
        gb_st = np.zeros((NSLOT, 2, D), np.float16)
        xTa_st = np.empty((RPC, 128, KC1, L), np.float16)
        for i, r in enumerate(rows):
            xTa_st[i] = _pm(x[r].T.astype(np.float16), KC1)

        table = []
        n_w2b_used = 0
        m2_seen = 0
        f8_seen = 0
        yidx = 0
        for s, kind in enumerate(kinds):
            if kind == "M2":
                row = rows[m2_seen % RPC]
                e = GEN if m2_seen < 2 else m2_jobs[row][1]
                g = 1.0 if m2_seen < 2 else m2_jobs[row][2]
                load = (m2_seen == 0) or (m2_seen == 2) or \
                    (m2_seen == 3 and m2_reload)
                if load:
                    w2q, sw2 = w8(2, e)
                    w2b_st[n_w2b_used] = w2q
                    n_w2b_used += 1
                else:
                    _, sw2 = w8(2, e)
                b1_st[:, s, :] = b1s[e].reshape(MC1, 128).T
                gam = float(gms[e].flat[0]) if fold else 1.0
                as_st[:, s, 1] = 1.0 / (g * gam) ** 2 if fold else 1.0
                as_st[:, s, 2] = LN_EPS * sw2 ** 2 / ((g * gam) ** 2 if fold else 1.0)
                gb_st[s, 0] = g * gms[e]
                gb_st[s, 1] = g * bts[e]
                xr = ((x[row] + b2s[e]) * sw2).astype(np.float16)
                xr_st[yidx] = xr[0:HL].reshape(2, 128, D).transpose(1, 0, 2)
                xr_st[yidx + 1] = xr[HL:L].reshape(2, 128, D).transpose(1, 0, 2)
                table.append((kind, row, e, None, False))
                m2_seen += 1
                yidx += 2
            else:
                f = f8_seen
                f8_seen += 1
                if f < len(f8_by_core[c]):
                    r, e, g, h = f8_by_core[c][f]
                    w1q, sw1 = w8(1, e)
                    w2q, sw2 = w8(2, e)
                    w1b_st[f] = w1q
                    w2b_st[n_w2b_used] = w2q
                    xh = x[r, h * HL:(h + 1) * HL]
                    sx = F8CAP / max(float(np.abs(xh).max()), 1e-30)
                    xtb_st[f] = _pm((xh.T * sx).astype(F8NP), KC1)
                    as_st[:, s, 0] = 1.0 / (sx * sw1)
                    gam = float(gms[e].flat[0]) if fold else 1.0
                    as_st[:, s, 1] = 1.0 / (g * gam) ** 2 if fold else 1.0
                    as_st[:, s, 2] = LN_EPS * sw2 ** 2 / ((g * gam) ** 2 if fold else 1.0)
                    b1_st[:, s, :] = b1s[e].reshape(MC1, 128).T
                    gb_st[s, 0] = g * gms[e]
                    gb_st[s, 1] = g * bts[e]
                    xr_st[yidx] = ((xh + b2s[e]) * sw2).astype(
                        np.float16).reshape(2, 128, D).transpose(1, 0, 2)
                    table.append((kind, r, e, h, False))
                else:
                    table.append((kind, None, None, None, True))
                n_w2b_used += 1
                yidx += 1
        slot_tables.append(table)
        in_maps.append({"w1a": w1a_st, "w1b": w1b_st, "w2b": w2b_st,
                        "xTa": xTa_st, "xTb": xtb_st, "xr": xr_st,
                        "b1": b1_st, "acts": as_st, "gb": gb_st})

    res = bass_utils.run_bass_kernel_spmd(nc, in_maps,
                                          core_ids=list(range(NCORES)))
    global last_run
    last_run = res

    # Combine: out[r] = y_general + bf16(sum of gated expert outputs).
    gen = np.zeros((B, L, D), np.float32)
    comb = np.zeros((B, L, D), np.float32)
    for c in range(NCORES):
        y = res.results[c]["y"].astype(np.float32)
        y = y.transpose(0, 2, 1, 3).reshape(-1, HL, D)
        yidx = 0
        m2_seen = 0
        for (kind, r, e, h, dummy) in slot_tables[c]:
            if kind == "M2":
                dst = gen if m2_seen < 2 else comb
                dst[r, 0:HL] += y[yidx]
                dst[r, HL:L] += y[yidx + 1]
                m2_seen += 1
                yidx += 2
            else:
                if not dummy:
                    comb[r, h * HL:(h + 1) * HL] += y[yidx]
                yidx += 1
    out = gen + comb.astype(ml_dtypes.bfloat16).astype(np.float32)
    return out


# revision 12
# speedup vs baseline: 1.0119x; 1.0119x over previous
"""Trainium2 Bass kernel for nn_IntraCycleMoELayer (MoE routing, 8 cores).

Strategy
--------
The reference computes all E=8 experts densely, but the top-2 gate zeroes all
but 2 experts per batch row.  Work that matters: per row, the top-1 routed
expert (gate >= 0.5 by softmax-renorm structure), the top-2 expert (often with
a near-zero gate), and the general expert, each an MLP block
  LN(gelu_tanh(x @ w1 + b1) @ w2 + b2 + x) * gamma + beta
over [L=512 tokens, D=768], DFF=3072.

Precision (for the 2e-2 rel-err budget; fp8e4 matmuls use DoubleRow perf
mode = 2x PE throughput, contracting K=256/instr):
  - general + top-1 expert ("M2"): mm1 fp16, gelu -> fp8 h, mm2 fp8
    DoubleRow.  The w2 absmax scale is folded into the residual (LN is
    scale-invariant up to eps, which is corrected via the Sqrt bias).
  - top-2 expert with gate >= 0.01 ("F8"): both matmuls fp8 DoubleRow; mm1
    dequant folds into the gelu activation's scale operand.
  - top-2 expert with gate < 0.01: skipped (contributes < 1e-3 rel).

When gamma is uniform and beta is zero (as in this model), the gate and
gamma fold into the LN rstd via the Sqrt activation's scale operand, so the
LN tail is a single tensor_scalar writing fp16 output.

Layout: core c owns rows 2c, 2c+1 -> 4 full M2 slots (general x2, top-1 x2);
the F8 jobs are split into 256-token halves spread 1 per core and run FIRST
(cheap fp8 weights shorten the critical head DMA); its mm2 is emitted after
the first M2 slot's mm1 so the gelu drain overlaps.  Host sums per-row
outputs (general + bf16(sum of routed)) exactly as the reference does.
"""
import numpy as np
import ml_dtypes

import concourse.bass as bass
import concourse.mybir as mybir
import concourse.tile as tile
from concourse import bacc
from concourse.bass import ts
from concourse import bass_utils

B, L, D, DFF, DLLM, E, TOPK = 16, 512, 768, 3072, 4096, 8, 2
EPS_GATE = 1e-9
LN_EPS = 1e-5
NCORES = 8
RPC = 2                               # rows per core
KC1, MC1 = D // 128, DFF // 128       # 6, 24
KC2, TC = DFF // 128, L // 128        # 24, 4
HL = L // 2                           # 256-token half
CB = DFF // 4                         # 768-column DMA piece
F8NP = ml_dtypes.float8_e4m3          # TRN float8e4 (max 240)
F8CAP = 240.0
SKIP_G = 0.01
dt = mybir.dt

_cache = {}   # (nf8, m2_reload, fold) -> finalized nc


def _router(cycle_numbers, DKP_embeddings, gate_We, gate_Wc, gate_b, gate_Wo,
            gate_bo):
    h = np.maximum(
        DKP_embeddings @ gate_We + cycle_numbers @ gate_Wc + gate_b, 0.0)
    logits = h @ gate_Wo + gate_bo                       # [B, E]
    idx = np.argsort(-logits, axis=1, kind="stable")[:, :TOPK]
    m = logits.max(axis=1, keepdims=True)
    p = np.exp(logits - m)
    p /= p.sum(axis=1, keepdims=True)
    mask = np.zeros_like(p)
    mask[np.arange(logits.shape[0])[:, None], idx] = 1.0
    gated = p * mask
    gated = gated / (gated.sum(axis=1, keepdims=True) + EPS_GATE)
    return idx, gated


def _slot_kinds(nf8, m2_reload):
    """Slot order shared by program build and host staging.

    Returns list of kinds: "F8" (half job, fp8 mm1+mm2) or "M2" (full job,
    fp16 mm1 + fp8 mm2).  w1a sets: [GEN, e6(row0), e6(row1) if reload].
    w2b sets are consumed in slot order by every slot's load.
    """
    kinds = []
    if nf8 >= 1:
        kinds.append("F8")
    kinds += ["M2", "M2", "M2", "M2"]
    kinds += ["F8"] * (nf8 - 1)
    return kinds


def _build_nc(nf8, m2_reload, fold):
    key = (nf8, m2_reload, fold)
    if key in _cache:
        return _cache[key]

    kinds = _slot_kinds(nf8, m2_reload)
    NSLOT = len(kinds)
    n_w1a = 2 + (1 if m2_reload else 0)
    # w2b sets in slot order: one per F8 slot, one for GEN (first M2),
    # one (or two with reload) for e6.
    n_w2b = nf8 + 2 + (1 if m2_reload else 0)
    n_w1b = max(nf8, 1)
    n_xtb = max(nf8, 1)
    NYH = 8 + nf8

    nc = bacc.Bacc("TRN2", target_bir_lowering=False, debug=False)
    # all staged pre-arranged partition-major so DMA lines are 3-18KB
    w1a_d = nc.dram_tensor("w1a", [n_w1a, 128, 4, KC1, CB], dt.float16, kind="ExternalInput")
    w1b_d = nc.dram_tensor("w1b", [n_w1b, 128, 4, KC1, CB], dt.float8e4, kind="ExternalInput")
    w2b_d = nc.dram_tensor("w2b", [n_w2b, 128, KC2, D], dt.float8e4, kind="ExternalInput")
    xTa_d = nc.dram_tensor("xTa", [RPC, 128, KC1, L], dt.float16, kind="ExternalInput")
    xTb_d = nc.dram_tensor("xTb", [n_xtb, 128, KC1, HL], dt.float8e4, kind="ExternalInput")
    xr_d = nc.dram_tensor("xr", [NYH, 128, 2, D], dt.float16, kind="ExternalInput")
    b1_d = nc.dram_tensor("b1", [128, NSLOT, MC1], dt.float32, kind="ExternalInput")
    as_d = nc.dram_tensor("acts", [128, NSLOT, 3], dt.float32, kind="ExternalInput")
    gb_d = nc.dram_tensor("gb", [NSLOT, 2, D], dt.float16, kind="ExternalInput")
    y_d = nc.dram_tensor("y", [NYH, 128, 2, D], dt.float16, kind="ExternalOutput")

    gelu = mybir.ActivationFunctionType.Gelu_apprx_tanh
    DR = mybir.MatmulPerfMode.DoubleRow

    with tile.TileContext(nc) as tc, \
         tc.tile_pool(name="w1ap", bufs=2) as w1ap, \
         tc.tile_pool(name="w1bp", bufs=1) as w1bp, \
         tc.tile_pool(name="w2bp", bufs=2) as w2bp, \
         tc.tile_pool(name="xtap", bufs=RPC) as xtap, \
         tc.tile_pool(name="xtbp", bufs=1) as xtbp, \
         tc.tile_pool(name="h8p", bufs=2) as h8p, \
         tc.tile_pool(name="xrp", bufs=3) as xrp, \
         tc.tile_pool(name="gbp", bufs=2) as gbp, \
         tc.tile_pool(name="rp", bufs=3) as rp, \
         tc.tile_pool(name="yp", bufs=3) as yp, \
         tc.tile_pool(name="sp", bufs=4) as sp, \
         tc.tile_pool(name="cp", bufs=1) as cp, \
         tc.tile_pool(name="php", bufs=4, space="PSUM") as php, \
         tc.tile_pool(name="pop", bufs=2, space="PSUM") as pop:

        # ---------- prologue: small loads + PE warmup ----------
        b1_all = cp.tile([128, NSLOT, MC1], dt.float32)
        nc.gpsimd.dma_start(b1_all, b1_d[:])
        as_all = cp.tile([128, NSLOT, 3], dt.float32)
        nc.gpsimd.dma_start(as_all, as_d[:])

        warm_z = cp.tile([128, 512], dt.float16)
        nc.vector.memset(warm_z, 0.0)
        for _ in range(16):
            wp_t = php.tile([128, 512], dt.float32, tag="ph")
            nc.tensor.matmul(wp_t, lhsT=warm_z[:, 0:128], rhs=warm_z,
                             start=True, stop=True)

        # ---------- prologue: critical-path weight streams ----------
        # sync queue: F8 weights first (small), then the second half of w1a
        # GEN + xT row1 + w1a e6 + w2b e6.
        # gpsimd queue: xT row0 + first half of w1a GEN + w2b e4/GEN.
        xtb_sb = None
        if nf8 >= 1:
            xtb_sb = xtbp.tile([128, KC1, HL], dt.float8e4, tag="xtb")
            nc.sync.dma_start(xtb_sb, xTb_d[0])
            w1b_sb = w1bp.tile([128, 4, KC1, CB], dt.float8e4, tag="w1b")
            for cb in range(0, 4, 2):
                nc.sync.dma_start(w1b_sb[:, cb:cb + 2], w1b_d[0][:, cb:cb + 2])

        xT_sb = []
        for _ in range(RPC):
            t_ = xtap.tile([128, KC1, L], dt.float16, tag="xT")
            xT_sb.append(t_)
        # scalar = second HWDGE queue; idle until the first gelu, so the
        # critical head weights ride both hardware queues in parallel.
        nc.scalar.dma_start(xT_sb[0], xTa_d[0])

        w1a_gen = w1ap.tile([128, 4, KC1, CB], dt.float16, tag="w1a")
        for cb in range(2):
            nc.scalar.dma_start(w1a_gen[:, cb], w1a_d[0][:, cb])
        for cb in range(2, 4):
            nc.sync.dma_start(w1a_gen[:, cb], w1a_d[0][:, cb])
        nc.sync.dma_start(xT_sb[1], xTa_d[1])

        w1a_e6 = [None, None]
        w1a_e6[0] = w1ap.tile([128, 4, KC1, CB], dt.float16, tag="w1a", name="w1a_e6a")
        for cb in range(0, 4, 2):
            nc.sync.dma_start(w1a_e6[0][:, cb:cb + 2], w1a_d[1][:, cb:cb + 2])
        if m2_reload:
            # Loaded lazily at its slot so the ring-WAR on the GEN buffer
            # doesn't block the sync queue head.
            w1a_e6[1] = None
        else:
            w1a_e6[1] = w1a_e6[0]

        # w2b sets, consumed in slot order.  e4 + GEN early on gpsimd; e6
        # late on sync (ring WAR on the e4 buffer resolves before needed).
        w2b_tiles = []
        n_w2b_emitted = 0

        def load_w2b(queue):
            nonlocal n_w2b_emitted
            t_ = w2bp.tile([128, KC2, D], dt.float8e4, tag="w2b")
            src = w2b_d[n_w2b_emitted]
            queue.dma_start(t_[:, 0:12, :], src[:, 0:12, :])
            queue.dma_start(t_[:, 12:KC2, :], src[:, 12:KC2, :])
            n_w2b_emitted += 1
            w2b_tiles.append(t_)
            return t_

        if nf8 >= 1:
            load_w2b(nc.scalar)            # e4 (slot 0)
        load_w2b(nc.sync)                  # GEN (M2 slots 0-1)
        load_w2b(nc.sync)                  # e6 (first M2e slot)
        if m2_reload:
            load_w2b(nc.sync)              # e6 second row

        kinds_ex = []   # (kind, slot_idx, yidx, row_or_f8idx, w1a_tile, w2b_tile)
        yidx = 0
        m2_seen = 0
        f8_seen = 0
        w2b_order = []
        if nf8 >= 1:
            w2b_order.append(0)
        w2b_order += [1 if nf8 >= 1 else 0] * 2
        base_e6 = (2 if nf8 >= 1 else 1)
        w2b_order += [base_e6, base_e6 + (1 if m2_reload else 0)]

        slot_descs = []
        for s, kind in enumerate(_slot_kinds(nf8, m2_reload)):
            if kind == "M2":
                row = m2_seen % RPC
                late = m2_seen == 3 and m2_reload
                w1a_t = w1a_gen if m2_seen < 2 else w1a_e6[row]
                w2b_t = w2b_tiles[w2b_order[s]]
                slot_descs.append(dict(kind=kind, s=s, yidx=yidx, row=row,
                                       w1a=w1a_t, w2b=w2b_t, late=late))
                m2_seen += 1
                yidx += 2
            else:
                slot_descs.append(dict(kind=kind, s=s, yidx=yidx, f8=f8_seen))
                f8_seen += 1
                yidx += 1

        # ---------- compute emission ----------
        def emit_mm1(sd):
            s = sd["s"]
            b1_sb = b1_all[:, s, :]
            gsc = as_all[:, s, 0:1]
            h8 = h8p.tile([128, KC2, L], dt.float8e4, tag="h8")
            sd["h8"] = h8
            if sd["kind"] == "F8":
                f = sd["f8"]
                if f == 0:
                    w1b_t, xtb_t = w1b_sb, xtb_sb
                else:
                    w1b_t = w1bp.tile([128, 4, KC1, CB], dt.float8e4, tag="w1b")
                    nc.sync.dma_start(w1b_t, w1b_d[f])
                    xtb_t = xtbp.tile([128, KC1, HL], dt.float8e4, tag="xtb")
                    nc.sync.dma_start(xtb_t, xTb_d[f])
                    sd["w2b"] = load_w2b(nc.gpsimd)
                for m in range(MC1):
                    ph = php.tile([128, 512], dt.float32, tag="ph")
                    cb, mc = m // 6, (m % 6) * 128
                    for k2 in range(KC1 // 2):
                        nc.tensor.matmul(
                            ph[:, 0:HL],
                            lhsT=w1b_t[:, cb, 2 * k2:2 * k2 + 2, mc:mc + 128],
                            rhs=xtb_t[:, 2 * k2:2 * k2 + 2, :],
                            start=(k2 == 0), stop=(k2 == KC1 // 2 - 1),
                            perf_mode=DR)
                    nc.scalar.activation(out=h8[:, m, 0:HL], in_=ph[:, 0:HL],
                                         func=gelu, bias=b1_sb[:, m:m + 1],
                                         scale=gsc)
            else:
                if sd.get("late") and sd["w1a"] is None:
                    w1a_t = w1ap.tile([128, 4, KC1, CB], dt.float16, tag="w1a")
                    for cb in range(0, 4, 2):
                        nc.sync.dma_start(w1a_t[:, cb:cb + 2],
                                          w1a_d[2][:, cb:cb + 2])
                    sd["w1a"] = w1a_t
                w1a_t = sd["w1a"]
                xT_t = xT_sb[sd["row"]]
                for m in range(MC1):
                    ph = php.tile([128, 512], dt.float32, tag="ph")
                    cb, mc = m // 6, (m % 6) * 128
                    for k in range(KC1):
                        nc.tensor.matmul(
                            ph, lhsT=w1a_t[:, cb, k, mc:mc + 128],
                            rhs=xT_t[:, k, :],
                            start=(k == 0), stop=(k == KC1 - 1))
                    nc.scalar.activation(out=h8[:, m, :], in_=ph,
                                         func=gelu, bias=b1_sb[:, m:m + 1],
                                         scale=gsc)

        def emit_mm2_ln(sd):
            s = sd["s"]
            h8 = sd["h8"]
            if sd["kind"] == "F8" and sd["f8"] == 0:
                w2b_t = w2b_tiles[0]
            else:
                w2b_t = sd["w2b"]
            nyh = 2 if sd["kind"] == "M2" else 1
            xr_sb = []
            for hh in range(nyh):
                t_ = xrp.tile([128, 2, D], dt.float16, tag="xr")
                nc.gpsimd.dma_start(t_, xr_d[sd["yidx"] + hh])
                xr_sb.append(t_)
            gb_sb = None
            if not fold:
                gb_sb = gbp.tile([128, 2, D], dt.float16, tag="gb")
                gb_ap = gb_d[s]
                nc.gpsimd.dma_start(gb_sb, bass.AP(tensor=gb_ap.tensor,
                                                   offset=gb_ap.offset,
                                                   ap=[[0, 128], *gb_ap.ap]))
            ntc = TC if sd["kind"] == "M2" else TC // 2
            for t in range(ntc):
                po = pop.tile([128, D], dt.float32, tag="po")
                for k2 in range(KC2 // 2):
                    nc.tensor.matmul(
                        po[:, 0:512],
                        lhsT=h8[:, 2 * k2:2 * k2 + 2, ts(t, 128)],
                        rhs=w2b_t[:, 2 * k2:2 * k2 + 2, 0:512],
                        start=(k2 == 0), stop=(k2 == KC2 // 2 - 1),
                        perf_mode=DR)
                    nc.tensor.matmul(
                        po[:, 512:D],
                        lhsT=h8[:, 2 * k2:2 * k2 + 2, ts(t, 128)],
                        rhs=w2b_t[:, 2 * k2:2 * k2 + 2, 512:D],
                        start=(k2 == 0), stop=(k2 == KC2 // 2 - 1),
                        perf_mode=DR)
                r_sb = rp.tile([128, D], dt.float32, tag="r")
                nc.vector.tensor_add(r_sb, po, xr_sb[t // 2][:, t % 2, :])
                stats = sp.tile([128, 3, 6], dt.float32, tag="st")
                for s3 in range(3):
                    nc.vector.bn_stats(stats[:, s3, :], r_sb[:, ts(s3, 256)])
                mv = sp.tile([128, 2], dt.float32, tag="mv")
                nc.vector.bn_aggr(mv, stats)
                rstd = sp.tile([128, 1], dt.float32, tag="rstd")
                nc.scalar.activation(out=rstd, in_=mv[:, 1:2],
                                     func=mybir.ActivationFunctionType.Sqrt,
                                     bias=as_all[:, s, 2:3],
                                     scale=as_all[:, s, 1:2])
                nc.vector.reciprocal(rstd, rstd)
                if t % 2 == 0:
                    yh16 = yp.tile([128, 2, D], dt.float16, tag="y16")
                y16 = yh16[:, t % 2, :]
                if fold:
                    nc.vector.tensor_scalar(out=y16, in0=r_sb,
                                            scalar1=mv[:, 0:1], scalar2=rstd,
                                            op0=mybir.AluOpType.subtract,
                                            op1=mybir.AluOpType.mult)
                else:
                    nc.vector.tensor_scalar(out=r_sb, in0=r_sb,
                                            scalar1=mv[:, 0:1], scalar2=rstd,
                                            op0=mybir.AluOpType.subtract,
                                            op1=mybir.AluOpType.mult)
                    nc.vector.tensor_mul(r_sb, r_sb, gb_sb[:, 0, :])
                    nc.vector.tensor_add(y16, r_sb, gb_sb[:, 1, :])
                if t % 2 == 1:
                    nc.sync.dma_start(y_d[sd["yidx"] + t // 2], yh16)

        if nf8 >= 1:
            # F8 mm1 first (cheap weights), then first M2 mm1 so the PE is
            # fed while the F8 gelu drains; F8 mm2 slots in after.
            emit_mm1(slot_descs[0])
            emit_mm1(slot_descs[1])
            emit_mm2_ln(slot_descs[0])
            emit_mm2_ln(slot_descs[1])
            rest = slot_descs[2:]
        else:
            rest = slot_descs
        for sd in rest:
            emit_mm1(sd)
            emit_mm2_ln(sd)

    nc.finalize()
    _cache[key] = nc
    return nc


def _pm(a, kchunks):
    """[K*128, N] -> [128, K, N] partition-major."""
    return np.ascontiguousarray(
        a.reshape(kchunks, 128, a.shape[-1]).transpose(1, 0, 2))


def _pm_w1(a):
    """[D, DFF] -> [128, 4(cb), KC1, CB] partition-major, cb-blocked."""
    return np.ascontiguousarray(
        a.reshape(KC1, 128, 4, CB).transpose(1, 2, 0, 3))


def kernel(cycle_curve_data, cycle_numbers, DKP_embeddings,
           gate_We, gate_Wc, gate_b, gate_Wo, gate_bo,
           e_w1, e_b1, e_w2, e_b2, e_gamma, e_beta,
           g_w1, g_b1, g_w2, g_b2, g_gamma, g_beta):
    x = np.asarray(cycle_curve_data, dtype=np.float32)
    idx, gated = _router(np.asarray(cycle_numbers, np.float32),
                         np.asarray(DKP_embeddings, np.float32),
                         np.asarray(gate_We, np.float32),
                         np.asarray(gate_Wc, np.float32),
                         np.asarray(gate_b, np.float32),
                         np.asarray(gate_Wo, np.float32),
                         np.asarray(gate_bo, np.float32))

    GEN = E
    w1s = {**{e: np.asarray(e_w1[e], np.float32) for e in range(E)},
           GEN: np.asarray(g_w1, np.float32)}
    w2s = {**{e: np.asarray(e_w2[e], np.float32) for e in range(E)},
           GEN: np.asarray(g_w2, np.float32)}
    b1s = {**{e: np.asarray(e_b1[e], np.float32) for e in range(E)},
           GEN: np.asarray(g_b1, np.float32)}
    b2s = {**{e: np.asarray(e_b2[e], np.float32) for e in range(E)},
           GEN: np.asarray(g_b2, np.float32)}
    gms = {**{e: np.asarray(e_gamma[e], np.float32) for e in range(E)},
           GEN: np.asarray(g_gamma, np.float32)}
    bts = {**{e: np.asarray(e_beta[e], np.float32) for e in range(E)},
           GEN: np.asarray(g_beta, np.float32)}

    # job classification (top-1 always has gate >= 0.5 -> M2; top-2 skipped
    # below SKIP_G, else F8 halves)
    m2_jobs = [(r, int(idx[r, 0]), float(gated[r, idx[r, 0]]))
               for r in range(B)]
    f8_jobs = [(r, int(idx[r, 1]), float(gated[r, idx[r, 1]]))
               for r in range(B) if gated[r, idx[r, 1]] >= SKIP_G]
    f8_halves = [(r, e, g, h) for (r, e, g) in f8_jobs for h in (0, 1)]
    nf8 = (len(f8_halves) + NCORES - 1) // NCORES
    m2_reload = any(m2_jobs[2 * c][1] != m2_jobs[2 * c + 1][1]
                    for c in range(NCORES))

    used_sets = {GEN} | {e for _, e, _ in m2_jobs} | {e for _, e, _ in f8_jobs}
    fold = all(
        np.all(gms[s] == gms[s].flat[0]) and gms[s].flat[0] > 0
        and np.all(bts[s] == 0.0) for s in used_sets)

    nc = _build_nc(nf8, m2_reload, fold)

    kinds = _slot_kinds(nf8, m2_reload)
    NSLOT = len(kinds)
    n_w1a = 2 + (1 if m2_reload else 0)
    n_w2b = nf8 + 2 + (1 if m2_reload else 0)
    n_w1b = max(nf8, 1)
    n_xtb = max(nf8, 1)
    NYH = 8 + nf8

    f16w, q8w = {}, {}

    def w16(s):
        if s not in f16w:
            f16w[s] = _pm_w1(w1s[s].astype(np.float16))
        return f16w[s]

    def w8(kind, s):
        if (kind, s) not in q8w:
            w = w1s[s] if kind == 1 else w2s[s]
            sc = F8CAP / max(float(np.abs(w).max()), 1e-30)
            q = (w * sc).astype(F8NP)
            q = _pm_w1(q) if kind == 1 else _pm(q, KC2)
            q8w[(kind, s)] = (q, sc)
        return q8w[(kind, s)]

    f8_by_core = [[] for _ in range(NCORES)]
    for i, hf in enumerate(f8_halves):
        f8_by_core[i % NCORES].append(hf)

    in_maps = []
    slot_tables = []   # per core: list of (kind, row, expert, half, dummy)
    for c in range(NCORES):
        rows = [RPC * c + i for i in range(RPC)]
        w1a_st = np.empty((n_w1a, 128, 4, KC1, CB), np.float16)
        w1a_st[0] = w16(GEN)
        w1a_st[1] = w16(m2_jobs[rows[0]][1])
        if m2_reload:
            w1a_st[2] = w16(m2_jobs[rows[1]][1])
        w2b_st = np.zeros((n_w2b, 128, KC2, D), F8NP)
        w1b_st = np.zeros((n_w1b, 128, 4, KC1, CB), F8NP)
        xtb_st = np.zeros((n_xtb, 128, KC1, HL), F8NP)
        xr_st = np.zeros((NYH, 128, 2, D), np.float16)
        b1_st = np.zeros((128, NSLOT, MC1), np.float32)
        as_st = np.ones((128, NSLOT, 3), np.float32)
        as_st[:, :, 2] = LN_EPS
        gb_st = np.zeros((NSLOT, 2, D), np.float16)
        xTa_st = np.empty((RPC, 128, KC1, L), np.float16)
        for i, r in enumerate(rows):
            xTa_st[i] = _pm(x[r].T.astype(np.float16), KC1)

        table = []
        n_w2b_used = 0
        m2_seen = 0
        f8_seen = 0
        yidx = 0
        for s, kind in enumerate(kinds):
            if kind == "M2":
                row = rows[m2_seen % RPC]
                e = GEN if m2_seen < 2 else m2_jobs[row][1]
                g = 1.0 if m2_seen < 2 else m2_jobs[row][2]
                load = (m2_seen == 0) or (m2_seen == 2) or \
                    (m2_seen == 3 and m2_reload)
                if load:
                    w2q, sw2 = w8(2, e)
                    w2b_st[n_w2b_used] = w2q
                    n_w2b_used += 1
                else:
                    _, sw2 = w8(2, e)
                b1_st[:, s, :] = b1s[e].reshape(MC1, 128).T
                gam = float(gms[e].flat[0]) if fold else 1.0
                as_st[:, s, 1] = 1.0 / (g * gam) ** 2 if fold else 1.0
                as_st[:, s, 2] = LN_EPS * sw2 ** 2 / ((g * gam) ** 2 if fold else 1.0)
                gb_st[s, 0] = g * gms[e]
                gb_st[s, 1] = g * bts[e]
                xr = ((x[row] + b2s[e]) * sw2).astype(np.float16)
                xr_st[yidx] = xr[0:HL].reshape(2, 128, D).transpose(1, 0, 2)
                xr_st[yidx + 1] = xr[HL:L].reshape(2, 128, D).transpose(1, 0, 2)
                table.append((kind, row, e, None, False))
                m2_seen += 1
                yidx += 2
            else:
                f = f8_seen
                f8_seen += 1
                if f < len(f8_by_core[c]):
                    r, e, g, h = f8_by_core[c][f]
                    w1q, sw1 = w8(1, e)
                    w2q, sw2 = w8(2, e)
                    w1b_st[f] = w1q
                    w2b_st[n_w2b_used] = w2q
                    xh = x[r, h * HL:(h + 1) * HL]
                    sx = F8CAP / max(float(np.abs(xh).max()), 1e-30)
                    xtb_st[f] = _pm((xh.T * sx).astype(F8NP), KC1)
                    as_st[:, s, 0] = 1.0 / (sx * sw1)
                    gam = float(gms[e].flat[0]) if fold else 1.0
                    as_st[:, s, 1] = 1.0 / (g * gam) ** 2 if fold else 1.0
                    as_st[:, s, 2] = LN_EPS * sw2 ** 2 / ((g * gam) ** 2 if fold else 1.0)
                    b1_st[:, s, :] = b1s[e].reshape(MC1, 128).T
                    gb_st[s, 0] = g * gms[e]
                    gb_st[s, 1] = g * bts[e]
                    xr_st[yidx] = ((xh + b2s[e]) * sw2).astype(
                        np.float16).reshape(2, 128, D).transpose(1, 0, 2)
                    table.append((kind, r, e, h, False))
                else:
                    table.append((kind, None, None, None, True))
                n_w2b_used += 1
                yidx += 1
        slot_tables.append(table)
        in_maps.append({"w1a": w1a_st, "w1b": w1b_st, "w2b": w2b_st,
                        "xTa": xTa_st, "xTb": xtb_st, "xr": xr_st,
                        "b1": b1_st, "acts": as_st, "gb": gb_st})

    res = bass_utils.run_bass_kernel_spmd(nc, in_maps,
                                          core_ids=list(range(NCORES)))
    global last_run
    last_run = res

    # Combine: out[r] = y_general + bf16(sum of gated expert outputs).
    gen = np.zeros((B, L, D), np.float32)
    comb = np.zeros((B, L, D), np.float32)
    for c in range(NCORES):
        y = res.results[c]["y"].astype(np.float32)
        y = y.transpose(0, 2, 1, 3).reshape(-1, HL, D)
        yidx = 0
        m2_seen = 0
        for (kind, r, e, h, dummy) in slot_tables[c]:
            if kind == "M2":
                dst = gen if m2_seen < 2 else comb
                dst[r, 0:HL] += y[yidx]
                dst[r, HL:L] += y[yidx + 1]
                m2_seen += 1
                yidx += 2
            else:
                if not dummy:
                    comb[r, h * HL:(h + 1) * HL] += y[yidx]
                yidx += 1
    out = gen + comb.astype(ml_dtypes.bfloat16).astype(np.float32)
    return out
